# revision 34
# baseline (speedup 1.0000x reference)
"""Distributed Trainium2 Bass kernel for nn_AMK_Block (kernelized-attention + ConvSwiGLU).

Sharding: sequence-parallel. Each of the 8 cores owns (batch b, query-row block q):
core = b*4 + q, rows q*512..q*512+511 of batch b, ALL heads. Each core computes
Q/K/V projections for its rows, AllGathers PhiK^T and V(+ones) across the 4 cores
of its batch group (bf16), then computes its 512 rows of attention, Wo, LN2 and
the full FFN locally. The depthwise-conv halo rows of Q_interact come from a tiny
third AllGather of boundary rows, extracted rank-agnostically with a mask-matrix
matmul. Matmuls run in bf16 (fp32 PSUM accumulation); norm/statistics in fp32.
"""

import sys

sys.path.insert(0, "/opt/trn_rl_repo")

from contextlib import ExitStack

import ml_dtypes
import numpy as np

import concourse.bass as bass
import concourse.tile as tile
from concourse import bacc, mybir
from concourse.bass_utils import run_bass_kernel_spmd
from concourse.masks import make_identity

F32 = mybir.dt.float32
F32R = mybir.dt.float32r
BF16 = mybir.dt.bfloat16
ALU = mybir.AluOpType
AF = mybir.ActivationFunctionType

B, N, D = 2, 2048, 1024
H, DH = 16, 64
INNER = 4096
LN_EPS = 1e-5

RO = 512          # owned rows per core
NQ = 4            # cores per batch group
GROUPS = [[0, 1, 2, 3], [4, 5, 6, 7]]
CHUNKS = [(0, 128), (128, 128), (256, 128), (384, 128)]
HALVES = [(0, 258), (258, 256)]  # even halves of 514 (fp32r/bf16 even free dim)

_cache: dict[float, object] = {}
_last_in_maps: list | None = None


def _build(dt_safe: float):
    nc = bacc.Bacc("TRN2", target_bir_lowering=False, debug=False, num_devices=8)

    # ---------------- DRAM parameters (per-core shapes) ----------------
    p_qin = nc.declare_dram_parameter("q_in", [RO, D], F32, isOutput=False)
    p_xb1 = nc.declare_dram_parameter("x_b1", [RO, D], F32, isOutput=False)
    # weight params are bf16 (host-converted): enables Fast Weight Load on the
    # PE and halves weight DMA traffic
    p_wq = nc.declare_dram_parameter("wq", [D, D], BF16, isOutput=False)
    p_wk = nc.declare_dram_parameter("wk", [D, D], BF16, isOutput=False)
    p_wv = nc.declare_dram_parameter("wv", [D, D], BF16, isOutput=False)
    p_wo = nc.declare_dram_parameter("wo", [D, D], BF16, isOutput=False)
    p_wup = nc.declare_dram_parameter("wup", [D, 2 * INNER], BF16, isOutput=False)
    p_bgu = nc.declare_dram_parameter("bias_gu", [2 * INNER], F32, isOutput=False)
    p_wd = nc.declare_dram_parameter("wdown", [INNER, D], BF16, isOutput=False)
    p_cw = nc.declare_dram_parameter("cw3", [INNER, 3], F32, isOutput=False)
    p_g1 = nc.declare_dram_parameter("g1", [D], F32, isOutput=False)
    p_mask = nc.declare_dram_parameter("masks", [2], F32, isOutput=False)
    p_mm = nc.declare_dram_parameter("maskmat", [2 * NQ, 2], F32R, isOutput=False)
    p_out = nc.declare_dram_parameter("out", [RO, D], F32, isOutput=True)

    with tile.TileContext(nc) as tc:
        build_ctx = ExitStack()
        with build_ctx:
            _emit(nc, tc, build_ctx, dt_safe, p_qin, p_xb1, p_wq, p_wk, p_wv,
                  p_wo, p_wup, p_bgu, p_wd, p_cw, p_g1, p_mask, p_mm, p_out)
    nc.finalize()
    return nc


def _emit(nc, tc, bctx, dt_safe, p_qin, p_xb1, p_wq, p_wk, p_wv, p_wo, p_wup,
          p_bgu, p_wd, p_cw, p_g1, p_mask, p_mm, p_out):
    # ---------------- constant tiles ----------------
    consts = bctx.enter_context(tc.tile_pool(name="consts", bufs=1))
    g1b = consts.tile([128, D], F32, name="g1b")
    nc.sync.dma_start(
        out=g1b[:],
        in_=bass.AP(tensor=p_g1, offset=0, ap=[[0, 128], [1, D]]),
    )
    ident_f = consts.tile([128, 128], F32, name="ident_f")
    make_identity(nc, ident_f[:])
    ident_r = consts.tile([128, 128], F32R, name="ident_r")
    nc.gpsimd.dma_start(out=ident_r[:], in_=ident_f[:])
    mask_p = consts.tile([128, 1], F32, name="mask_p")
    nc.sync.dma_start(out=mask_p[:], in_=bass.AP(tensor=p_mask, offset=0, ap=[[0, 128], [1, 1]]))
    mask_n = consts.tile([128, 1], F32, name="mask_n")
    nc.sync.dma_start(out=mask_n[:], in_=bass.AP(tensor=p_mask, offset=1, ap=[[0, 128], [1, 1]]))
    ones_col = consts.tile([128, 1], F32, name="ones_col")
    nc.vector.memset(ones_col[:], 1.0)
    eps_t = consts.tile([128, 1], F32, name="eps_t")
    nc.vector.memset(eps_t[:], LN_EPS)
    maskmat = consts.tile([2 * NQ, 2], F32R, name="maskmat")
    nc.sync.dma_start(out=maskmat[:], in_=p_mm[:, :])

    # DRAM scratch for the collectives
    dram = bctx.enter_context(tc.tile_pool(name="dram", bufs=1, space="DRAM"))
    kag_in = dram.tile([H * DH, RO], BF16, name="kag_in")      # PhiK^T local slice
    kag1 = dram.tile([NQ * 512, RO], BF16, name="kag1")        # gathered heads 0-7
    kag2 = dram.tile([NQ * 512, RO], BF16, name="kag2")        # gathered heads 8-15
    vag_in = dram.tile([RO, H * 65], BF16, name="vag_in")      # V(+ones) local rows
    vag1 = dram.tile([NQ * 256, H * 65], BF16, name="vag1")    # gathered rows 0-255
    vag2 = dram.tile([NQ * 256, H * 65], BF16, name="vag2")    # gathered rows 256-511
    hag_in = dram.tile([2, D], F32R, name="hag_in")            # my boundary Qint rows
    hag = dram.tile([2 * NQ, D], F32R, name="hag")             # gathered boundaries

    ev_state = [0]

    def evict_copy(dst_ap, src_ap):
        ev_state[0] += 1
        if ev_state[0] % 2 == 0:
            nc.vector.tensor_copy(dst_ap, src_ap)
        else:
            nc.scalar.activation(dst_ap, src_ap, AF.Copy)

    ln_pool = bctx.enter_context(tc.tile_pool(name="ln", bufs=3))

    def layernorm_to(x_ap, p):
        """Returns (mv, rstd) tiles: mean in mv[:,0:1], rstd [p,1], for x_ap [p, D]."""
        st = ln_pool.tile([128, 2, 6], F32, tag="bn_st")
        xr = x_ap.rearrange("p (s f) -> p s f", s=2)
        for s in range(2):
            nc.vector.bn_stats(st[:p, s, :], xr[:, s, :])
        mv = ln_pool.tile([128, 2], F32, tag="bn_mv")
        nc.vector.bn_aggr(mv[:p], st[:p])
        rstd = ln_pool.tile([128, 1], F32, tag="bn_rstd")
        nc.scalar.activation(rstd[:p], mv[:p, 1:2], AF.Sqrt, bias=eps_t[:p, 0:1])
        nc.vector.reciprocal(rstd[:p], rstd[:p])
        return mv, rstd

    # ---- lifetime stacks (must nest LIFO): f34 > av2 > av > phase stacks ----
    f34_stack = ExitStack()   # hfc + qint: from Wo until the end
    av2_stack = ExitStack()   # mTc: until end of Wo
    av_stack = ExitStack()    # vT, phiQ: until end of head loop
    hfc_pool = f34_stack.enter_context(tc.tile_pool(name="hfc", bufs=1))
    qint_pool = f34_stack.enter_context(tc.tile_pool(name="qint", bufs=1))
    mTc_pool = av2_stack.enter_context(tc.tile_pool(name="mTc", bufs=1))
    vT_pool = av_stack.enter_context(tc.tile_pool(name="vT", bufs=1))
    phiQ_pool = av_stack.enter_context(tc.tile_pool(name="phiQ", bufs=1))
    mTc = [mTc_pool.tile([128, RO], BF16, name=f"mTc{j}") for j in range(8)]
    vT = [vT_pool.tile([128, RO], F32R, name=f"vT{j}") for j in range(8)]
    phiQT = [phiQ_pool.tile([128, RO], BF16, name=f"phiQT{j}") for j in range(8)]

    # ---------------- Phase P: LN1 + Hc + transposes ----------------
    p_stack = ExitStack()
    hcT_pool = p_stack.enter_context(tc.tile_pool(name="hcT", bufs=1))
    hcT = [hcT_pool.tile([128, RO], BF16, name=f"hcT{j}") for j in range(8)]
    io_pool = p_stack.enter_context(tc.tile_pool(name="io", bufs=2))
    hc_pool = p_stack.enter_context(tc.tile_pool(name="hc", bufs=2))
    pp_t = p_stack.enter_context(tc.tile_pool(name="pp_t", bufs=2, space="PSUM"))

    for i, (r0, p) in enumerate(CHUNKS):
        qin_t = io_pool.tile([p, D], F32, tag="qin")
        nc.sync.dma_start(out=qin_t[:], in_=p_qin[r0:r0 + p, :])
        xb1_t = io_pool.tile([p, D], F32, tag="xb1")
        nc.sync.dma_start(out=xb1_t[:], in_=p_xb1[r0:r0 + p, :])

        mv, rstd = layernorm_to(qin_t[:p, :], p)
        hc_t = hc_pool.tile([p, D], F32, tag="hc")
        nc.vector.tensor_scalar(
            out=hc_t[:p, :], in0=qin_t[:p, :], scalar1=mv[:p, 0:1],
            scalar2=rstd[:p, 0:1], op0=ALU.subtract, op1=ALU.mult,
        )
        nc.vector.tensor_mul(hc_t[:p, :], hc_t[:p, :], g1b[:p, :])
        nc.vector.tensor_add(hc_t[:p, :], hc_t[:p, :], xb1_t[:p, :])

        # transpose this row-chunk into the 8 hcT column tiles
        for j in range(8):
            tp = pp_t.tile([128, 128], F32, tag="tp")
            nc.tensor.transpose(tp[:128, :p], hc_t[:p, j * 128:(j + 1) * 128], ident_f[:p, :p])
            evict_copy(hcT[j][:, r0:r0 + p], tp[:128, :p])

    # ---------------- Phase P: projections ----------------
    # Order: K -> K-AllGather (smallest latency to first collective), then Q
    # (needed with K for the W matmuls), then V -> V-AllGather. The rings
    # serialize on the collective lane, so K's goes first.
    wstream = p_stack.enter_context(tc.tile_pool(name="wstream", bufs=8))
    pp_a = p_stack.enter_context(tc.tile_pool(name="pp_a", bufs=4, space="PSUM"))
    elu_pool = p_stack.enter_context(tc.tile_pool(name="elu", bufs=2))

    def elu1_evict(dst_ap, src_psum_ap, p, w):
        """dst = elu(src)+1 = relu(src) + exp(min(src,0))"""
        tmin = elu_pool.tile([128, 512], F32, tag="tmin")
        nc.vector.tensor_scalar_min(tmin[:p, :w], src_psum_ap, 0.0)
        texp = elu_pool.tile([128, 512], F32, tag="texp")
        nc.scalar.activation(texp[:p, :w], tmin[:p, :w], AF.Exp)
        nc.vector.scalar_tensor_tensor(
            out=dst_ap, in0=src_psum_ap, scalar=0.0, in1=texp[:p, :w],
            op0=ALU.max, op1=ALU.add,
        )

    # K^T -> PhiK^T -> kag_in; two chunked AllGathers (heads 0-7, 8-15) so the
    # attention can start on the first half while the second is in flight
    wk_sb = []
    for k in range(8):
        w_t = wstream.tile([128, D], BF16, tag="wproj")
        nc.sync.dma_start(out=w_t[:], in_=p_wk[k * 128:(k + 1) * 128, :])
        wk_sb.append(w_t)
    phiK_pool = p_stack.enter_context(tc.tile_pool(name="phiK", bufs=3))
    for j in range(8):
        ps = pp_a.tile([128, 512], F32, tag="proj")
        for k in range(8):
            nc.tensor.matmul(
                ps[:], wk_sb[k][:, j * 128:(j + 1) * 128],
                hcT[k][:, 0:RO], start=(k == 0), stop=(k == 7),
            )
        phiK_t = phiK_pool.tile([128, RO], BF16, tag="phiK")
        elu1_evict(phiK_t[:, :], ps[:], 128, RO)
        nc.sync.dma_start(out=kag_in[j * 128:(j + 1) * 128, :], in_=phiK_t[:])
        if j == 3:
            nc.gpsimd.collective_compute(
                "AllGather", ALU.bypass, replica_groups=GROUPS,
                ins=[kag_in[0:512, :].opt()], outs=[kag1[:].opt()],
            )
    nc.gpsimd.collective_compute(
        "AllGather", ALU.bypass, replica_groups=GROUPS,
        ins=[kag_in[512:1024, :].opt()], outs=[kag2[:].opt()],
    )

    # Q^T -> PhiQ^T (kept in SBUF)
    wq_sb = []
    for k in range(8):
        w_t = wstream.tile([128, D], BF16, tag="wproj")
        nc.sync.dma_start(out=w_t[:], in_=p_wq[k * 128:(k + 1) * 128, :])
        wq_sb.append(w_t)
    for j in range(8):
        ps = pp_a.tile([128, 512], F32, tag="proj")
        for k in range(8):
            nc.tensor.matmul(
                ps[:], wq_sb[k][:, j * 128:(j + 1) * 128],
                hcT[k][:, 0:RO], start=(k == 0), stop=(k == 7),
            )
        elu1_evict(phiQT[j][:, :], ps[:], 128, RO)

    # V^T, then transpose back to row-major (+ones cols) and stage for its AGs
    wv_sb = []
    for k in range(8):
        w_t = wstream.tile([128, D], BF16, tag="wproj")
        nc.sync.dma_start(out=w_t[:], in_=p_wv[k * 128:(k + 1) * 128, :])
        wv_sb.append(w_t)
    for j in range(8):
        ps = pp_a.tile([128, 512], F32, tag="proj")
        for k in range(8):
            nc.tensor.matmul(
                ps[:], wv_sb[k][:, j * 128:(j + 1) * 128],
                hcT[k][:, 0:RO], start=(k == 0), stop=(k == 7),
            )
        evict_copy(vT[j][:, :], ps[:])

    vs_pool = p_stack.enter_context(tc.tile_pool(name="vs", bufs=2))
    for i in range(4):
        r0 = i * 128
        vstage = vs_pool.tile([128, H * 65], BF16, tag="vstage")
        for j in range(8):
            tpv = pp_t.tile([128, 128], F32R, tag="tpv")
            nc.tensor.transpose(tpv[:], vT[j][:, r0:r0 + 128], ident_r[:])
            h0, h1 = 2 * j, 2 * j + 1
            evict_copy(vstage[:, h0 * 65:h0 * 65 + 64], tpv[:, 0:64])
            evict_copy(vstage[:, h1 * 65:h1 * 65 + 64], tpv[:, 64:128])
        for h in range(H):
            nc.vector.tensor_copy(vstage[:, h * 65 + 64:h * 65 + 65], ones_col[:])
        nc.sync.dma_start(out=vag_in[r0:r0 + 128, :], in_=vstage[:])
        if i == 1:
            nc.gpsimd.collective_compute(
                "AllGather", ALU.bypass, replica_groups=GROUPS,
                ins=[vag_in[0:256, :].opt()], outs=[vag1[:].opt()],
            )
    nc.gpsimd.collective_compute(
        "AllGather", ALU.bypass, replica_groups=GROUPS,
        ins=[vag_in[256:512, :].opt()], outs=[vag2[:].opt()],
    )

    p_stack.close()

    # ---------------- Phase A: attention ----------------
    a_stack = ExitStack()
    kq_pool = a_stack.enter_context(tc.tile_pool(name="kq", bufs=8))
    vhd_pool = a_stack.enter_context(tc.tile_pool(name="vhd", bufs=24))
    wt_pool = a_stack.enter_context(tc.tile_pool(name="wt", bufs=48))
    asm_pool = a_stack.enter_context(tc.tile_pool(name="asm", bufs=4))
    pp_w = a_stack.enter_context(tc.tile_pool(name="pp_w", bufs=6, space="PSUM"))
    pp_at = a_stack.enter_context(tc.tile_pool(name="pp_at", bufs=2, space="PSUM"))

    sq_state = [0]

    def square_evict(dst_ap, src_psum_ap, tmp_pool, w):
        """dst = src^2 (W = PhiQ.PhiK^T > 0, so relu is a no-op).
        ACT does it in one pass, DVE needs two; weight 2 ACT : 1 DVE."""
        sq_state[0] += 1
        if sq_state[0] % 3 < 2:
            nc.scalar.square(dst_ap, src_psum_ap)
        else:
            tr = tmp_pool.tile([128, 512], F32, tag="r2tmpv")
            nc.vector.tensor_copy(tr[:, :w], src_psum_ap)
            nc.vector.tensor_mul(dst_ap, tr[:, :w], tr[:, :w])

    # m-chunk order follows the chunked V gathers: vag1 rows first
    M_ORDER = [m for m in range(16) if m % 4 < 2] + [m for m in range(16) if m % 4 >= 2]

    def vag_slice(m, h):
        qq, lc = m // 4, m % 4
        if lc < 2:
            return vag1[qq * 256 + lc * 128: qq * 256 + (lc + 1) * 128,
                        h * 65:(h + 1) * 65]
        return vag2[qq * 256 + (lc - 2) * 128: qq * 256 + (lc - 1) * 128,
                    h * 65:(h + 1) * 65]

    for h in range(H):
        j2, off = h // 2, (h % 2) * 64
        kag_t = kag1 if h < 8 else kag2
        hrow = (h % 8) * DH
        kq_sb = []
        for qq in range(NQ):
            kt = kq_pool.tile([128, RO], BF16, tag="kq")
            nc.sync.dma_start(
                out=kt[off:off + 64, :],
                in_=kag_t[qq * 512 + hrow: qq * 512 + hrow + DH, :],
            )
            kq_sb.append(kt)
        # W^T chunks, squared
        wt_sb = {}
        for m in range(16):
            qq, lc = m // 4, m % 4
            wt_t = wt_pool.tile([128, RO], BF16, tag="wt")
            psw = pp_w.tile([128, 512], F32, tag="psw")
            nc.tensor.matmul(
                psw[:], kq_sb[qq][off:off + 64, lc * 128:(lc + 1) * 128],
                phiQT[j2][off:off + 64, :], start=True, stop=True,
            )
            square_evict(wt_t[:, :], psw[:], asm_pool, RO)
            wt_sb[m] = wt_t
        # AttrT (+ row 64 = sum for Norm): accumulate over the 16 m-chunks
        pat = pp_at.tile([65, 512], F32, tag="pat", name=f"pat{h}")
        for mi, m in enumerate(M_ORDER):
            vhd = vhd_pool.tile([128, 65], BF16, tag="vhd")
            nc.scalar.dma_start(out=vhd[:], in_=vag_slice(m, h))
            nc.tensor.matmul(
                pat[:], vhd[:], wt_sb[m][:, :],
                start=(mi == 0), stop=(mi == 15),
            )
        # Norm -> fast reciprocal -> broadcast; mT = AttrT * (1/Norm) - V^T
        nrm = asm_pool.tile([1, RO], F32, tag="nrm")
        nc.vector.tensor_scalar_add(nrm[0:1, :], pat[64:65, :], 1.0)
        nc.vector.reciprocal_approx_fast(out=nrm[:], in_=nrm[:])
        rcb = asm_pool.tile([64, RO], F32, tag="rcb")
        nc.gpsimd.partition_broadcast(rcb[:], nrm[:])
        tm = asm_pool.tile([128, RO], F32, tag="tm")
        nc.vector.tensor_mul(tm[off:off + 64, :], pat[0:64, :], rcb[:, :])
        nc.vector.tensor_sub(
            mTc[j2][off:off + 64, :], tm[off:off + 64, :], vT[j2][off:off + 64, :],
        )
    a_stack.close()
    av_stack.close()  # frees vT, phiQ

    # ---------------- Phase A5: Wo + Q_interact ----------------
    a5_stack = ExitStack()
    wo_pool = a5_stack.enter_context(tc.tile_pool(name="wo", bufs=8))
    qi_pool = a5_stack.enter_context(tc.tile_pool(name="qi", bufs=3))
    pp_o = a5_stack.enter_context(tc.tile_pool(name="pp_o", bufs=4, space="PSUM"))
    wo_sb = []
    for k in range(8):
        w_t = wo_pool.tile([128, D], BF16, tag="wo")
        nc.sync.dma_start(out=w_t[:], in_=p_wo[k * 128:(k + 1) * 128, :])
        wo_sb.append(w_t)
    qint = []
    for i, (r0, p) in enumerate(CHUNKS):
        qin_t = qi_pool.tile([p, D], F32, tag="qin2")
        nc.sync.dma_start(out=qin_t[:], in_=p_qin[r0:r0 + p, :])
        qi = qint_pool.tile([p, D], F32, name=f"qint{i}")
        for half in range(2):
            pso = pp_o.tile([128, 512], F32, tag="pso")
            for k in range(8):
                nc.tensor.matmul(
                    pso[:p, :], mTc[k][:, r0:r0 + p],
                    wo_sb[k][:, half * 512:(half + 1) * 512],
                    start=(k == 0), stop=(k == 7),
                )
            nc.vector.scalar_tensor_tensor(
                out=qi[:p, half * 512:(half + 1) * 512], in0=pso[:p, :],
                scalar=dt_safe, in1=qin_t[:p, half * 512:(half + 1) * 512],
                op0=ALU.mult, op1=ALU.add,
            )
        qint.append(qi)

    # stage my boundary Q_interact rows and AllGather them (conv halo exchange)
    nc.gpsimd.dma_start(out=hag_in[0:1, :], in_=qint[0][0:1, :])
    nc.gpsimd.dma_start(out=hag_in[1:2, :], in_=qint[3][127:128, :])
    nc.gpsimd.collective_compute(
        "AllGather", ALU.bypass, replica_groups=GROUPS,
        ins=[hag_in[:].opt()], outs=[hag[:].opt()],
    )
    a5_stack.close()
    av2_stack.close()  # frees mTc

    # ---------------- Phase F: LN2 + transpose + FFN ----------------
    hfc = []

    f_stack = ExitStack()
    qn2_pool = f_stack.enter_context(tc.tile_pool(name="qn2", bufs=2))
    pp_f = f_stack.enter_context(tc.tile_pool(name="pp_f", bufs=2, space="PSUM"))
    pp_g = f_stack.enter_context(tc.tile_pool(name="pp_g", bufs=3, space="PSUM"))
    pp_u = f_stack.enter_context(tc.tile_pool(name="pp_u", bufs=3, space="PSUM"))
    qn2T_pool = f_stack.enter_context(tc.tile_pool(name="qn2T", bufs=1))
    # qn2T cols: 0..511 owned rows, 512 = prev-halo row, 513 = next-halo row
    qn2T = [qn2T_pool.tile([128, RO + 2], BF16, name=f"qn2T{j}") for j in range(8)]
    for i, (r0, p) in enumerate(CHUNKS):
        mv, rstd = layernorm_to(qint[i][:p, :], p)
        qn2_t = qn2_pool.tile([p, D], F32, tag="qn2")
        nc.vector.tensor_scalar(
            out=qn2_t[:p, :], in0=qint[i][:p, :], scalar1=mv[:p, 0:1],
            scalar2=rstd[:p, 0:1], op0=ALU.subtract, op1=ALU.mult,
        )
        for j in range(8):
            tp = pp_f.tile([128, 128], F32, tag="tpf")
            nc.tensor.transpose(tp[:128, :p], qn2_t[:p, j * 128:(j + 1) * 128], ident_f[:p, :p])
            evict_copy(qn2T[j][:, r0:r0 + p], tp[:128, :p])

    # halo rows: extract prev/next boundary rows via maskmat.T @ gathered,
    # then LN2 + transpose into qn2T cols 512/513
    hg_sb = qn2_pool.tile([2 * NQ, D], F32R, name="hg_sb")
    nc.sync.dma_start(out=hg_sb[:], in_=hag[:, :])
    qih = qn2_pool.tile([2, D], F32, name="qih")
    for half in range(2):
        ph = pp_g.tile([2, 512], F32, tag="psg", name=f"ph{half}")
        nc.tensor.matmul(
            ph[:], maskmat[:], hg_sb[:, half * 512:(half + 1) * 512],
            start=True, stop=True,
        )
        nc.vector.tensor_copy(qih[:, half * 512:(half + 1) * 512], ph[:])
    mv, rstd = layernorm_to(qih[:2, :], 2)
    qn2h = qn2_pool.tile([2, D], F32, name="qn2h")
    nc.vector.tensor_scalar(
        out=qn2h[:2, :], in0=qih[:2, :], scalar1=mv[:2, 0:1],
        scalar2=rstd[:2, 0:1], op0=ALU.subtract, op1=ALU.mult,
    )
    for j in range(8):
        tp = pp_f.tile([128, 128], F32, tag="tpf")
        nc.tensor.transpose(tp[:128, :2], qn2h[:2, j * 128:(j + 1) * 128], ident_f[:2, :2])
        evict_copy(qn2T[j][:, RO:RO + 2], tp[:128, :2])

    # Wup + SwiGLU + depthwise conv, in 512-col superchunks
    wup_pool = f_stack.enter_context(tc.tile_pool(name="wup", bufs=8))
    wupu_pool = f_stack.enter_context(tc.tile_pool(name="wupu", bufs=8))
    fsm_pool = f_stack.enter_context(tc.tile_pool(name="fsm", bufs=3))
    bias_pool = f_stack.enter_context(tc.tile_pool(name="bias", bufs=6))

    for sc in range(8):
        wupg_sb, wupu_sb = [], []
        for k in range(8):
            wg = wup_pool.tile([128, 512], BF16, tag="wupg")
            nc.sync.dma_start(
                out=wg[:], in_=p_wup[k * 128:(k + 1) * 128, sc * 512:(sc + 1) * 512]
            )
            wupg_sb.append(wg)
            wu = wupu_pool.tile([128, 512], BF16, tag="wupu")
            nc.sync.dma_start(
                out=wu[:], in_=p_wup[k * 128:(k + 1) * 128, INNER + sc * 512:INNER + (sc + 1) * 512]
            )
            wupu_sb.append(wu)
        for c in range(4):
            cc = sc * 4 + c
            bg = bias_pool.tile([128, 1], F32, tag="bg")
            nc.sync.dma_start(out=bg[:], in_=bass.AP(tensor=p_bgu, offset=cc * 128, ap=[[1, 128], [1, 1]]))
            bu = bias_pool.tile([128, 1], F32, tag="bu")
            nc.sync.dma_start(out=bu[:], in_=bass.AP(tensor=p_bgu, offset=INNER + cc * 128, ap=[[1, 128], [1, 1]]))
            cw = bias_pool.tile([128, 3], F32, tag="cw")
            nc.sync.dma_start(out=cw[:], in_=p_cw[cc * 128:(cc + 1) * 128, :])

            gact = fsm_pool.tile([128, RO + 2], F32, tag="gact")
            for h0, w in HALVES:
                psg = pp_g.tile([128, 258], F32, tag="psg")
                for k in range(8):
                    nc.tensor.matmul(
                        psg[:, :w], wupg_sb[k][:, c * 128:(c + 1) * 128],
                        qn2T[k][:, h0:h0 + w], start=(k == 0), stop=(k == 7),
                    )
                nc.scalar.activation(gact[:, h0:h0 + w], psg[:, :w], AF.Silu, bias=bg[:, 0:1])
            hf = fsm_pool.tile([128, RO + 2], F32, tag="hf")
            for h0, w in HALVES:
                psu = pp_u.tile([128, 258], F32, tag="psu")
                for k in range(8):
                    nc.tensor.matmul(
                        psu[:, :w], wupu_sb[k][:, c * 128:(c + 1) * 128],
                        qn2T[k][:, h0:h0 + w], start=(k == 0), stop=(k == 7),
                    )
                nc.vector.scalar_tensor_tensor(
                    out=hf[:, h0:h0 + w], in0=psu[:, :w], scalar=bu[:, 0:1],
                    in1=gact[:, h0:h0 + w], op0=ALU.add, op1=ALU.mult,
                )
            # mask halo cols at batch edges (conv zero-pad)
            nc.vector.tensor_scalar_mul(hf[:, RO:RO + 1], hf[:, RO:RO + 1], mask_p[:, 0:1])
            nc.vector.tensor_scalar_mul(hf[:, RO + 1:RO + 2], hf[:, RO + 1:RO + 2], mask_n[:, 0:1])
            # depthwise conv along rows: cols 0..511 owned, 512=prev, 513=next.
            # center tap on ACT (scale is per-partition), side taps DVE/GpSimd.
            hfc_t = hfc_pool.tile([128, RO], BF16, name=f"hfc{cc}")
            nc.scalar.activation(hfc_t[:, 0:RO], hf[:, 0:RO], AF.Copy, scale=cw[:, 1:2])
            nc.vector.scalar_tensor_tensor(
                out=hfc_t[:, 1:RO], in0=hf[:, 0:RO - 1], scalar=cw[:, 0:1],
                in1=hfc_t[:, 1:RO], op0=ALU.mult, op1=ALU.add,
            )
            nc.vector.scalar_tensor_tensor(
                out=hfc_t[:, 0:1], in0=hf[:, RO:RO + 1], scalar=cw[:, 0:1],
                in1=hfc_t[:, 0:1], op0=ALU.mult, op1=ALU.add,
            )
            nc.vector.scalar_tensor_tensor(
                out=hfc_t[:, 0:RO - 1], in0=hf[:, 1:RO], scalar=cw[:, 2:3],
                in1=hfc_t[:, 0:RO - 1], op0=ALU.mult, op1=ALU.add,
            )
            nc.vector.scalar_tensor_tensor(
                out=hfc_t[:, RO - 1:RO], in0=hf[:, RO + 1:RO + 2], scalar=cw[:, 2:3],
                in1=hfc_t[:, RO - 1:RO], op0=ALU.mult, op1=ALU.add,
            )
            hfc.append(hfc_t)
    f_stack.close()

    # ---------------- Phase F4: Wdown + residual + output ----------------
    f4_stack = ExitStack()
    wd_pool = f4_stack.enter_context(tc.tile_pool(name="wd", bufs=4))
    out_pool = f4_stack.enter_context(tc.tile_pool(name="outp", bufs=4))
    pp_d = f4_stack.enter_context(tc.tile_pool(name="pp_d", bufs=1, space="PSUM"))
    psd = [[pp_d.tile([128, 512], F32, name=f"psd{i}_{half}") for half in range(2)]
           for i in range(4)]
    for k in range(INNER // 128):
        wd_t = wd_pool.tile([128, D], BF16, tag="wd")
        nc.sync.dma_start(out=wd_t[:], in_=p_wd[k * 128:(k + 1) * 128, :])
        for i in range(4):
            for half in range(2):
                nc.tensor.matmul(
                    psd[i][half][:], hfc[k][:, i * 128:(i + 1) * 128],
                    wd_t[:, half * 512:(half + 1) * 512],
                    start=(k == 0), stop=(k == INNER // 128 - 1),
                )
    for i in range(4):
        o_t = out_pool.tile([128, D], F32, tag="osb")
        for half in range(2):
            s = slice(half * 512, (half + 1) * 512)
            nc.vector.tensor_add(o_t[:, s], psd[i][half][:], qint[i][:, s])
        nc.sync.dma_start(out=p_out[i * 128:(i + 1) * 128, :], in_=o_t[:])
    f4_stack.close()
    f34_stack.close()


def kernel(**inputs) -> np.ndarray:
    Q_in = np.ascontiguousarray(np.asarray(inputs["Q_in"], dtype=np.float32))
    X = np.ascontiguousarray(np.asarray(inputs["X"], dtype=np.float32))
    Wq = np.asarray(inputs["Wq"], dtype=np.float32)
    Wk = np.asarray(inputs["Wk"], dtype=np.float32)
    Wv = np.asarray(inputs["Wv"], dtype=np.float32)
    Wo = np.asarray(inputs["Wo"], dtype=np.float32)
    Wup = np.asarray(inputs["Wup"], dtype=np.float32)
    conv_w = np.asarray(inputs["conv_w"], dtype=np.float32)
    Wdown = np.asarray(inputs["Wdown"], dtype=np.float32)
    g1 = np.asarray(inputs["g1"], dtype=np.float32)
    b1 = np.asarray(inputs["b1"], dtype=np.float32)
    g2 = np.asarray(inputs["g2"], dtype=np.float32)
    b2 = np.asarray(inputs["b2"], dtype=np.float32)
    dt = float(np.asarray(inputs["dt"], dtype=np.float32))

    # softplus(dt) on host; baked into the NEFF as an immediate
    dt_safe = float(np.log1p(np.exp(dt)))

    # fold g2/b2 into Wup (LN2's affine commutes into the up-projection)
    wup_f = g2[:, None] * Wup
    bias_gu = np.ascontiguousarray(b2 @ Wup)
    cw3 = np.ascontiguousarray(conv_w[:, 0, :])

    key = round(dt_safe, 9)
    if key not in _cache:
        _cache[key] = _build(dt_safe)
    nc = _cache[key]

    bf = ml_dtypes.bfloat16
    wq_b = np.ascontiguousarray(Wq.astype(bf))
    wk_b = np.ascontiguousarray(Wk.astype(bf))
    wv_b = np.ascontiguousarray(Wv.astype(bf))
    wo_b = np.ascontiguousarray(Wo.astype(bf))
    wup_b = np.ascontiguousarray(wup_f.astype(bf))
    wd_b = np.ascontiguousarray(Wdown.astype(bf))

    in_maps = []
    for core in range(8):
        b, q = divmod(core, 4)
        qin = np.ascontiguousarray(Q_in[b, q * RO:(q + 1) * RO])
        xb1 = np.ascontiguousarray(X[b, q * RO:(q + 1) * RO] + b1[None, :])
        masks = np.array(
            [1.0 if q > 0 else 0.0, 1.0 if q < NQ - 1 else 0.0], dtype=np.float32
        )
        # maskmat.T @ gathered_boundaries = [prev-halo row; next-halo row]
        mm = np.zeros((2 * NQ, 2), dtype=np.float32)
        if q > 0:
            mm[2 * (q - 1) + 1, 0] = 1.0
        if q < NQ - 1:
            mm[2 * (q + 1), 1] = 1.0
        in_maps.append({
            "q_in": qin, "x_b1": xb1, "wq": wq_b, "wk": wk_b, "wv": wv_b,
            "wo": wo_b, "wup": wup_b, "bias_gu": bias_gu, "wdown": wd_b,
            "cw3": cw3, "g1": np.ascontiguousarray(g1), "masks": masks,
            "maskmat": mm,
        })

    global _last_in_maps
    _last_in_maps = in_maps
    res = run_bass_kernel_spmd(nc, in_maps, core_ids=list(range(8)))

    out = np.empty((B, N, D), dtype=np.float32)
    for core in range(8):
        b, q = divmod(core, 4)
        out[b, q * RO:(q + 1) * RO] = res.results[core]["out"]
    return out


# revision 36
# speedup vs baseline: 1.0313x; 1.0313x over previous
"""Distributed Trainium2 Bass kernel for nn_AMK_Block (kernelized-attention + ConvSwiGLU).

Sharding: sequence-parallel. Each of the 8 cores owns (batch b, query-row block q):
core = b*4 + q, rows q*512..q*512+511 of batch b, ALL heads. Each core computes
Q/K/V projections for its rows, AllGathers PhiK^T and V(+ones) across the 4 cores
of its batch group (bf16), then computes its 512 rows of attention, Wo, LN2 and
the full FFN locally. The depthwise-conv halo rows of Q_interact come from a tiny
third AllGather of boundary rows, extracted rank-agnostically with a mask-matrix
matmul. Matmuls run in bf16 (fp32 PSUM accumulation); norm/statistics in fp32.
"""

import sys

sys.path.insert(0, "/opt/trn_rl_repo")

from contextlib import ExitStack

import ml_dtypes
import numpy as np

import concourse.bass as bass
import concourse.tile as tile
from concourse import bacc, mybir
from concourse.bass_utils import run_bass_kernel_spmd
from concourse.masks import make_identity

F32 = mybir.dt.float32
F32R = mybir.dt.float32r
BF16 = mybir.dt.bfloat16
ALU = mybir.AluOpType
AF = mybir.ActivationFunctionType

B, N, D = 2, 2048, 1024
H, DH = 16, 64
INNER = 4096
LN_EPS = 1e-5

RO = 512          # owned rows per core
NQ = 4            # cores per batch group
GROUPS = [[0, 1, 2, 3], [4, 5, 6, 7]]
CHUNKS = [(0, 128), (128, 128), (256, 128), (384, 128)]
HALVES = [(0, 258), (258, 256)]  # even halves of 514 (fp32r/bf16 even free dim)

_cache: dict[float, object] = {}
_last_in_maps: list | None = None


def _build(dt_safe: float):
    nc = bacc.Bacc("TRN2", target_bir_lowering=False, debug=False, num_devices=8)

    # ---------------- DRAM parameters (per-core shapes) ----------------
    p_qin = nc.declare_dram_parameter("q_in", [RO, D], F32, isOutput=False)
    p_xb1 = nc.declare_dram_parameter("x_b1", [RO, D], F32, isOutput=False)
    # weight params are bf16 (host-converted): enables Fast Weight Load on the
    # PE and halves weight DMA traffic
    p_wq = nc.declare_dram_parameter("wq", [D, D], BF16, isOutput=False)
    p_wk = nc.declare_dram_parameter("wk", [D, D], BF16, isOutput=False)
    p_wv = nc.declare_dram_parameter("wv", [D, D], BF16, isOutput=False)
    p_wo = nc.declare_dram_parameter("wo", [D, D], BF16, isOutput=False)
    p_wup = nc.declare_dram_parameter("wup", [D, 2 * INNER], BF16, isOutput=False)
    p_bgu = nc.declare_dram_parameter("bias_gu", [2 * INNER], F32, isOutput=False)
    p_wd = nc.declare_dram_parameter("wdown", [INNER, D], BF16, isOutput=False)
    p_cw = nc.declare_dram_parameter("cw3", [INNER, 3], F32, isOutput=False)
    p_g1 = nc.declare_dram_parameter("g1", [D], F32, isOutput=False)
    p_mask = nc.declare_dram_parameter("masks", [2], F32, isOutput=False)
    p_mm = nc.declare_dram_parameter("maskmat", [2 * NQ, 2], F32R, isOutput=False)
    p_out = nc.declare_dram_parameter("out", [RO, D], F32, isOutput=True)

    with tile.TileContext(nc) as tc:
        build_ctx = ExitStack()
        with build_ctx:
            _emit(nc, tc, build_ctx, dt_safe, p_qin, p_xb1, p_wq, p_wk, p_wv,
                  p_wo, p_wup, p_bgu, p_wd, p_cw, p_g1, p_mask, p_mm, p_out)
    nc.finalize()
    return nc


def _emit(nc, tc, bctx, dt_safe, p_qin, p_xb1, p_wq, p_wk, p_wv, p_wo, p_wup,
          p_bgu, p_wd, p_cw, p_g1, p_mask, p_mm, p_out):
    # ---------------- constant tiles ----------------
    consts = bctx.enter_context(tc.tile_pool(name="consts", bufs=1))
    g1b = consts.tile([128, D], F32, name="g1b")
    nc.sync.dma_start(
        out=g1b[:],
        in_=bass.AP(tensor=p_g1, offset=0, ap=[[0, 128], [1, D]]),
    )
    ident_f = consts.tile([128, 128], F32, name="ident_f")
    make_identity(nc, ident_f[:])
    ident_r = consts.tile([128, 128], F32R, name="ident_r")
    nc.gpsimd.dma_start(out=ident_r[:], in_=ident_f[:])
    mask_p = consts.tile([128, 1], F32, name="mask_p")
    nc.sync.dma_start(out=mask_p[:], in_=bass.AP(tensor=p_mask, offset=0, ap=[[0, 128], [1, 1]]))
    mask_n = consts.tile([128, 1], F32, name="mask_n")
    nc.sync.dma_start(out=mask_n[:], in_=bass.AP(tensor=p_mask, offset=1, ap=[[0, 128], [1, 1]]))
    ones_col = consts.tile([128, 1], F32, name="ones_col")
    nc.vector.memset(ones_col[:], 1.0)
    eps_t = consts.tile([128, 1], F32, name="eps_t")
    nc.vector.memset(eps_t[:], LN_EPS)
    maskmat = consts.tile([2 * NQ, 2], F32R, name="maskmat")
    nc.sync.dma_start(out=maskmat[:], in_=p_mm[:, :])

    # DRAM scratch for the collectives
    dram = bctx.enter_context(tc.tile_pool(name="dram", bufs=1, space="DRAM"))
    kag_in = dram.tile([H * DH, RO], BF16, name="kag_in")      # PhiK^T local slice
    kag1 = dram.tile([NQ * 512, RO], BF16, name="kag1")        # gathered heads 0-7
    kag2 = dram.tile([NQ * 512, RO], BF16, name="kag2")        # gathered heads 8-15
    vag_in = dram.tile([RO, H * 65], BF16, name="vag_in")      # V(+ones) local rows
    vag1 = dram.tile([NQ * 256, H * 65], BF16, name="vag1")    # gathered rows 0-255
    vag2 = dram.tile([NQ * 256, H * 65], BF16, name="vag2")    # gathered rows 256-511
    hag_in = dram.tile([2, D], F32R, name="hag_in")            # my boundary Qint rows
    hag = dram.tile([2 * NQ, D], F32R, name="hag")             # gathered boundaries

    ev_state = [0]

    def evict_copy(dst_ap, src_ap):
        ev_state[0] += 1
        if ev_state[0] % 2 == 0:
            nc.vector.tensor_copy(dst_ap, src_ap)
        else:
            nc.scalar.activation(dst_ap, src_ap, AF.Copy)

    ln_pool = bctx.enter_context(tc.tile_pool(name="ln", bufs=3))

    def layernorm_to(x_ap, p):
        """Returns (mv, rstd) tiles: mean in mv[:,0:1], rstd [p,1], for x_ap [p, D]."""
        st = ln_pool.tile([128, 2, 6], F32, tag="bn_st")
        xr = x_ap.rearrange("p (s f) -> p s f", s=2)
        for s in range(2):
            nc.vector.bn_stats(st[:p, s, :], xr[:, s, :])
        mv = ln_pool.tile([128, 2], F32, tag="bn_mv")
        nc.vector.bn_aggr(mv[:p], st[:p])
        rstd = ln_pool.tile([128, 1], F32, tag="bn_rstd")
        nc.scalar.activation(rstd[:p], mv[:p, 1:2], AF.Sqrt, bias=eps_t[:p, 0:1])
        nc.vector.reciprocal(rstd[:p], rstd[:p])
        return mv, rstd

    # ---- lifetime stacks (must nest LIFO): f34 > av2 > av > phase stacks ----
    f34_stack = ExitStack()   # hfc + qint: from Wo until the end
    av2_stack = ExitStack()   # mTc: until end of Wo
    av_stack = ExitStack()    # vT, phiQ: until end of head loop
    hfc_pool = f34_stack.enter_context(tc.tile_pool(name="hfc", bufs=1))
    qint_pool = f34_stack.enter_context(tc.tile_pool(name="qint", bufs=1))
    mTc_pool = av2_stack.enter_context(tc.tile_pool(name="mTc", bufs=1))
    vT_pool = av_stack.enter_context(tc.tile_pool(name="vT", bufs=1))
    phiQ_pool = av_stack.enter_context(tc.tile_pool(name="phiQ", bufs=1))
    mTc = [mTc_pool.tile([128, RO], BF16, name=f"mTc{j}") for j in range(8)]
    vT = [vT_pool.tile([128, RO], F32R, name=f"vT{j}") for j in range(8)]
    phiQT = [phiQ_pool.tile([128, RO], BF16, name=f"phiQT{j}") for j in range(8)]

    # ---------------- Phase P: LN1 + Hc + transposes ----------------
    p_stack = ExitStack()
    hcT_pool = p_stack.enter_context(tc.tile_pool(name="hcT", bufs=1))
    hcT = [hcT_pool.tile([128, RO], BF16, name=f"hcT{j}") for j in range(8)]
    io_pool = p_stack.enter_context(tc.tile_pool(name="io", bufs=2))
    hc_pool = p_stack.enter_context(tc.tile_pool(name="hc", bufs=2))
    pp_t = p_stack.enter_context(tc.tile_pool(name="pp_t", bufs=2, space="PSUM"))

    for i, (r0, p) in enumerate(CHUNKS):
        qin_t = io_pool.tile([p, D], F32, tag="qin")
        nc.sync.dma_start(out=qin_t[:], in_=p_qin[r0:r0 + p, :])
        xb1_t = io_pool.tile([p, D], F32, tag="xb1")
        nc.sync.dma_start(out=xb1_t[:], in_=p_xb1[r0:r0 + p, :])

        mv, rstd = layernorm_to(qin_t[:p, :], p)
        hc_t = hc_pool.tile([p, D], F32, tag="hc")
        nc.vector.tensor_scalar(
            out=hc_t[:p, :], in0=qin_t[:p, :], scalar1=mv[:p, 0:1],
            scalar2=rstd[:p, 0:1], op0=ALU.subtract, op1=ALU.mult,
        )
        nc.vector.tensor_mul(hc_t[:p, :], hc_t[:p, :], g1b[:p, :])
        nc.vector.tensor_add(hc_t[:p, :], hc_t[:p, :], xb1_t[:p, :])

        # transpose this row-chunk into the 8 hcT column tiles
        for j in range(8):
            tp = pp_t.tile([128, 128], F32, tag="tp")
            nc.tensor.transpose(tp[:128, :p], hc_t[:p, j * 128:(j + 1) * 128], ident_f[:p, :p])
            evict_copy(hcT[j][:, r0:r0 + p], tp[:128, :p])

    # ---------------- Phase P: projections ----------------
    # Order: K -> K-AllGather (smallest latency to first collective), then Q
    # (needed with K for the W matmuls), then V -> V-AllGather. The rings
    # serialize on the collective lane, so K's goes first.
    wstream = p_stack.enter_context(tc.tile_pool(name="wstream", bufs=8))
    pp_a = p_stack.enter_context(tc.tile_pool(name="pp_a", bufs=4, space="PSUM"))
    elu_pool = p_stack.enter_context(tc.tile_pool(name="elu", bufs=2))

    def elu1_evict(dst_ap, src_psum_ap, p, w):
        """dst = elu(src)+1 = relu(src) + exp(min(src,0))"""
        tmin = elu_pool.tile([128, 512], F32, tag="tmin")
        nc.vector.tensor_scalar_min(tmin[:p, :w], src_psum_ap, 0.0)
        texp = elu_pool.tile([128, 512], F32, tag="texp")
        nc.scalar.activation(texp[:p, :w], tmin[:p, :w], AF.Exp)
        nc.vector.scalar_tensor_tensor(
            out=dst_ap, in0=src_psum_ap, scalar=0.0, in1=texp[:p, :w],
            op0=ALU.max, op1=ALU.add,
        )

    # K^T -> PhiK^T -> kag_in; two chunked AllGathers (heads 0-7, 8-15) so the
    # attention can start on the first half while the second is in flight
    wk_sb = []
    for k in range(8):
        w_t = wstream.tile([128, D], BF16, tag="wproj")
        nc.sync.dma_start(out=w_t[:], in_=p_wk[k * 128:(k + 1) * 128, :])
        wk_sb.append(w_t)
    phiK_pool = p_stack.enter_context(tc.tile_pool(name="phiK", bufs=3))
    for j in range(8):
        ps = pp_a.tile([128, 512], F32, tag="proj")
        for k in range(8):
            nc.tensor.matmul(
                ps[:], wk_sb[k][:, j * 128:(j + 1) * 128],
                hcT[k][:, 0:RO], start=(k == 0), stop=(k == 7),
            )
        phiK_t = phiK_pool.tile([128, RO], BF16, tag="phiK")
        elu1_evict(phiK_t[:, :], ps[:], 128, RO)
        nc.sync.dma_start(out=kag_in[j * 128:(j + 1) * 128, :], in_=phiK_t[:])
        if j == 3:
            nc.gpsimd.collective_compute(
                "AllGather", ALU.bypass, replica_groups=GROUPS,
                ins=[kag_in[0:512, :].opt()], outs=[kag1[:].opt()],
            )
    nc.gpsimd.collective_compute(
        "AllGather", ALU.bypass, replica_groups=GROUPS,
        ins=[kag_in[512:1024, :].opt()], outs=[kag2[:].opt()],
    )

    # Q^T -> PhiQ^T (kept in SBUF)
    wq_sb = []
    for k in range(8):
        w_t = wstream.tile([128, D], BF16, tag="wproj")
        nc.sync.dma_start(out=w_t[:], in_=p_wq[k * 128:(k + 1) * 128, :])
        wq_sb.append(w_t)
    for j in range(8):
        ps = pp_a.tile([128, 512], F32, tag="proj")
        for k in range(8):
            nc.tensor.matmul(
                ps[:], wq_sb[k][:, j * 128:(j + 1) * 128],
                hcT[k][:, 0:RO], start=(k == 0), stop=(k == 7),
            )
        elu1_evict(phiQT[j][:, :], ps[:], 128, RO)

    # V^T, then transpose back to row-major (+ones cols) and stage for its AGs
    wv_sb = []
    for k in range(8):
        w_t = wstream.tile([128, D], BF16, tag="wproj")
        nc.sync.dma_start(out=w_t[:], in_=p_wv[k * 128:(k + 1) * 128, :])
        wv_sb.append(w_t)
    for j in range(8):
        ps = pp_a.tile([128, 512], F32, tag="proj")
        for k in range(8):
            nc.tensor.matmul(
                ps[:], wv_sb[k][:, j * 128:(j + 1) * 128],
                hcT[k][:, 0:RO], start=(k == 0), stop=(k == 7),
            )
        evict_copy(vT[j][:, :], ps[:])

    vs_pool = p_stack.enter_context(tc.tile_pool(name="vs", bufs=2))
    for i in range(4):
        r0 = i * 128
        vstage = vs_pool.tile([128, H * 65], BF16, tag="vstage")
        for j in range(8):
            tpv = pp_t.tile([128, 128], F32R, tag="tpv")
            nc.tensor.transpose(tpv[:], vT[j][:, r0:r0 + 128], ident_r[:])
            h0, h1 = 2 * j, 2 * j + 1
            evict_copy(vstage[:, h0 * 65:h0 * 65 + 64], tpv[:, 0:64])
            evict_copy(vstage[:, h1 * 65:h1 * 65 + 64], tpv[:, 64:128])
        for h in range(H):
            nc.vector.tensor_copy(vstage[:, h * 65 + 64:h * 65 + 65], ones_col[:])
        nc.sync.dma_start(out=vag_in[r0:r0 + 128, :], in_=vstage[:])
        if i == 1:
            nc.gpsimd.collective_compute(
                "AllGather", ALU.bypass, replica_groups=GROUPS,
                ins=[vag_in[0:256, :].opt()], outs=[vag1[:].opt()],
            )
    nc.gpsimd.collective_compute(
        "AllGather", ALU.bypass, replica_groups=GROUPS,
        ins=[vag_in[256:512, :].opt()], outs=[vag2[:].opt()],
    )

    p_stack.close()

    # ---------------- Phase A: attention ----------------
    a_stack = ExitStack()
    kq_pool = a_stack.enter_context(tc.tile_pool(name="kq", bufs=8))
    vhd_pool = a_stack.enter_context(tc.tile_pool(name="vhd", bufs=24))
    wt_pool = a_stack.enter_context(tc.tile_pool(name="wt", bufs=48))
    asm_pool = a_stack.enter_context(tc.tile_pool(name="asm", bufs=4))
    pp_w = a_stack.enter_context(tc.tile_pool(name="pp_w", bufs=4, space="PSUM"))
    pp_at = a_stack.enter_context(tc.tile_pool(name="pp_at", bufs=4, space="PSUM"))

    sq_state = [0]

    def square_evict(dst_ap, src_psum_ap, tmp_pool, w):
        """dst = src^2 (W = PhiQ.PhiK^T > 0, so relu is a no-op).
        ACT does it in one pass, DVE needs two; weight 2 ACT : 1 DVE."""
        sq_state[0] += 1
        if sq_state[0] % 3 < 2:
            nc.scalar.square(dst_ap, src_psum_ap)
        else:
            tr = tmp_pool.tile([128, 512], F32, tag="r2tmpv")
            nc.vector.tensor_copy(tr[:, :w], src_psum_ap)
            nc.vector.tensor_mul(dst_ap, tr[:, :w], tr[:, :w])

    # m-chunk order follows the chunked V gathers: vag1 rows first
    M_ORDER = [m for m in range(16) if m % 4 < 2] + [m for m in range(16) if m % 4 >= 2]

    def vag_slice(m, h):
        qq, lc = m // 4, m % 4
        if lc < 2:
            return vag1[qq * 256 + lc * 128: qq * 256 + (lc + 1) * 128,
                        h * 65:(h + 1) * 65]
        return vag2[qq * 256 + (lc - 2) * 128: qq * 256 + (lc - 1) * 128,
                    h * 65:(h + 1) * 65]

    for hg in range(H // 4):
        pats = []
        for hh in range(4):
            h = hg * 4 + hh
            j2, off = h // 2, (h % 2) * 64
            kag_t = kag1 if h < 8 else kag2
            hrow = (h % 8) * DH
            kq_sb = []
            for qq in range(NQ):
                kt = kq_pool.tile([128, RO], BF16, tag="kq")
                nc.sync.dma_start(
                    out=kt[off:off + 64, :],
                    in_=kag_t[qq * 512 + hrow: qq * 512 + hrow + DH, :],
                )
                kq_sb.append(kt)
            # W^T chunks, squared
            wt_sb = {}
            for m in range(16):
                qq, lc = m // 4, m % 4
                wt_t = wt_pool.tile([128, RO], BF16, tag="wt")
                psw = pp_w.tile([128, 512], F32, tag="psw")
                nc.tensor.matmul(
                    psw[:], kq_sb[qq][off:off + 64, lc * 128:(lc + 1) * 128],
                    phiQT[j2][off:off + 64, :], start=True, stop=True,
                )
                square_evict(wt_t[:, :], psw[:], asm_pool, RO)
                wt_sb[m] = wt_t
            # AttrT (+ row 64 = sum for Norm): accumulate over the 16 m-chunks
            pat = pp_at.tile([65, 512], F32, tag="pat", name=f"pat{h}")
            for mi, m in enumerate(M_ORDER):
                vhd = vhd_pool.tile([128, 65], BF16, tag="vhd")
                nc.scalar.dma_start(out=vhd[:], in_=vag_slice(m, h))
                nc.tensor.matmul(
                    pat[:], vhd[:], wt_sb[m][:, :],
                    start=(mi == 0), stop=(mi == 15),
                )
            pats.append(pat)
        for hh in range(4):
            h = hg * 4 + hh
            j2, off = h // 2, (h % 2) * 64
            nrm = asm_pool.tile([1, RO], F32, tag="nrm")
            nc.vector.tensor_scalar_add(nrm[0:1, :], pats[hh][64:65, :], 1.0)
            nc.vector.reciprocal_approx_fast(out=nrm[:], in_=nrm[:])
            rcb = asm_pool.tile([64, RO], F32, tag="rcb")
            nc.gpsimd.partition_broadcast(rcb[:], nrm[:])
            tm = asm_pool.tile([128, RO], F32, tag="tm")
            nc.vector.tensor_mul(tm[off:off + 64, :], pats[hh][0:64, :], rcb[:, :])
            nc.vector.tensor_sub(
                mTc[j2][off:off + 64, :], tm[off:off + 64, :],
                vT[j2][off:off + 64, :],
            )
    a_stack.close()
    av_stack.close()  # frees vT, phiQ

    # ---------------- Phase A5: Wo + Q_interact ----------------
    a5_stack = ExitStack()
    wo_pool = a5_stack.enter_context(tc.tile_pool(name="wo", bufs=8))
    qi_pool = a5_stack.enter_context(tc.tile_pool(name="qi", bufs=3))
    pp_o = a5_stack.enter_context(tc.tile_pool(name="pp_o", bufs=4, space="PSUM"))
    wo_sb = []
    for k in range(8):
        w_t = wo_pool.tile([128, D], BF16, tag="wo")
        nc.sync.dma_start(out=w_t[:], in_=p_wo[k * 128:(k + 1) * 128, :])
        wo_sb.append(w_t)
    qint = []
    for i, (r0, p) in enumerate(CHUNKS):
        qin_t = qi_pool.tile([p, D], F32, tag="qin2")
        nc.sync.dma_start(out=qin_t[:], in_=p_qin[r0:r0 + p, :])
        qi = qint_pool.tile([p, D], F32, name=f"qint{i}")
        for half in range(2):
            pso = pp_o.tile([128, 512], F32, tag="pso")
            for k in range(8):
                nc.tensor.matmul(
                    pso[:p, :], mTc[k][:, r0:r0 + p],
                    wo_sb[k][:, half * 512:(half + 1) * 512],
                    start=(k == 0), stop=(k == 7),
                )
            nc.vector.scalar_tensor_tensor(
                out=qi[:p, half * 512:(half + 1) * 512], in0=pso[:p, :],
                scalar=dt_safe, in1=qin_t[:p, half * 512:(half + 1) * 512],
                op0=ALU.mult, op1=ALU.add,
            )
        qint.append(qi)

    # stage my boundary Q_interact rows and AllGather them (conv halo exchange)
    nc.gpsimd.dma_start(out=hag_in[0:1, :], in_=qint[0][0:1, :])
    nc.gpsimd.dma_start(out=hag_in[1:2, :], in_=qint[3][127:128, :])
    nc.gpsimd.collective_compute(
        "AllGather", ALU.bypass, replica_groups=GROUPS,
        ins=[hag_in[:].opt()], outs=[hag[:].opt()],
    )
    a5_stack.close()
    av2_stack.close()  # frees mTc

    # ---------------- Phase F: LN2 + transpose + FFN ----------------
    hfc = []

    f_stack = ExitStack()
    qn2_pool = f_stack.enter_context(tc.tile_pool(name="qn2", bufs=2))
    pp_f = f_stack.enter_context(tc.tile_pool(name="pp_f", bufs=2, space="PSUM"))
    pp_g = f_stack.enter_context(tc.tile_pool(name="pp_g", bufs=3, space="PSUM"))
    pp_u = f_stack.enter_context(tc.tile_pool(name="pp_u", bufs=3, space="PSUM"))
    qn2T_pool = f_stack.enter_context(tc.tile_pool(name="qn2T", bufs=1))
    # qn2T cols: 0..511 owned rows, 512 = prev-halo row, 513 = next-halo row
    qn2T = [qn2T_pool.tile([128, RO + 2], BF16, name=f"qn2T{j}") for j in range(8)]
    for i, (r0, p) in enumerate(CHUNKS):
        mv, rstd = layernorm_to(qint[i][:p, :], p)
        qn2_t = qn2_pool.tile([p, D], F32, tag="qn2")
        nc.vector.tensor_scalar(
            out=qn2_t[:p, :], in0=qint[i][:p, :], scalar1=mv[:p, 0:1],
            scalar2=rstd[:p, 0:1], op0=ALU.subtract, op1=ALU.mult,
        )
        for j in range(8):
            tp = pp_f.tile([128, 128], F32, tag="tpf")
            nc.tensor.transpose(tp[:128, :p], qn2_t[:p, j * 128:(j + 1) * 128], ident_f[:p, :p])
            evict_copy(qn2T[j][:, r0:r0 + p], tp[:128, :p])

    # halo rows: extract prev/next boundary rows via maskmat.T @ gathered,
    # then LN2 + transpose into qn2T cols 512/513
    hg_sb = qn2_pool.tile([2 * NQ, D], F32R, name="hg_sb")
    nc.sync.dma_start(out=hg_sb[:], in_=hag[:, :])
    qih = qn2_pool.tile([2, D], F32, name="qih")
    for half in range(2):
        ph = pp_g.tile([2, 512], F32, tag="psg", name=f"ph{half}")
        nc.tensor.matmul(
            ph[:], maskmat[:], hg_sb[:, half * 512:(half + 1) * 512],
            start=True, stop=True,
        )
        nc.vector.tensor_copy(qih[:, half * 512:(half + 1) * 512], ph[:])
    mv, rstd = layernorm_to(qih[:2, :], 2)
    qn2h = qn2_pool.tile([2, D], F32, name="qn2h")
    nc.vector.tensor_scalar(
        out=qn2h[:2, :], in0=qih[:2, :], scalar1=mv[:2, 0:1],
        scalar2=rstd[:2, 0:1], op0=ALU.subtract, op1=ALU.mult,
    )
    for j in range(8):
        tp = pp_f.tile([128, 128], F32, tag="tpf")
        nc.tensor.transpose(tp[:128, :2], qn2h[:2, j * 128:(j + 1) * 128], ident_f[:2, :2])
        evict_copy(qn2T[j][:, RO:RO + 2], tp[:128, :2])

    # Wup + SwiGLU + depthwise conv, in 512-col superchunks
    wup_pool = f_stack.enter_context(tc.tile_pool(name="wup", bufs=8))
    wupu_pool = f_stack.enter_context(tc.tile_pool(name="wupu", bufs=8))
    fsm_pool = f_stack.enter_context(tc.tile_pool(name="fsm", bufs=3))
    bias_pool = f_stack.enter_context(tc.tile_pool(name="bias", bufs=6))

    for sc in range(8):
        wupg_sb, wupu_sb = [], []
        for k in range(8):
            wg = wup_pool.tile([128, 512], BF16, tag="wupg")
            nc.sync.dma_start(
                out=wg[:], in_=p_wup[k * 128:(k + 1) * 128, sc * 512:(sc + 1) * 512]
            )
            wupg_sb.append(wg)
            wu = wupu_pool.tile([128, 512], BF16, tag="wupu")
            nc.sync.dma_start(
                out=wu[:], in_=p_wup[k * 128:(k + 1) * 128, INNER + sc * 512:INNER + (sc + 1) * 512]
            )
            wupu_sb.append(wu)
        for c in range(4):
            cc = sc * 4 + c
            bg = bias_pool.tile([128, 1], F32, tag="bg")
            nc.sync.dma_start(out=bg[:], in_=bass.AP(tensor=p_bgu, offset=cc * 128, ap=[[1, 128], [1, 1]]))
            bu = bias_pool.tile([128, 1], F32, tag="bu")
            nc.sync.dma_start(out=bu[:], in_=bass.AP(tensor=p_bgu, offset=INNER + cc * 128, ap=[[1, 128], [1, 1]]))
            cw = bias_pool.tile([128, 3], F32, tag="cw")
            nc.sync.dma_start(out=cw[:], in_=p_cw[cc * 128:(cc + 1) * 128, :])

            gact = fsm_pool.tile([128, RO + 2], F32, tag="gact")
            for h0, w in HALVES:
                psg = pp_g.tile([128, 258], F32, tag="psg")
                for k in range(8):
                    nc.tensor.matmul(
                        psg[:, :w], wupg_sb[k][:, c * 128:(c + 1) * 128],
                        qn2T[k][:, h0:h0 + w], start=(k == 0), stop=(k == 7),
                    )
                nc.scalar.activation(gact[:, h0:h0 + w], psg[:, :w], AF.Silu, bias=bg[:, 0:1])
            hf = fsm_pool.tile([128, RO + 2], F32, tag="hf")
            for h0, w in HALVES:
                psu = pp_u.tile([128, 258], F32, tag="psu")
                for k in range(8):
                    nc.tensor.matmul(
                        psu[:, :w], wupu_sb[k][:, c * 128:(c + 1) * 128],
                        qn2T[k][:, h0:h0 + w], start=(k == 0), stop=(k == 7),
                    )
                nc.vector.scalar_tensor_tensor(
                    out=hf[:, h0:h0 + w], in0=psu[:, :w], scalar=bu[:, 0:1],
                    in1=gact[:, h0:h0 + w], op0=ALU.add, op1=ALU.mult,
                )
            # mask halo cols at batch edges (conv zero-pad)
            nc.vector.tensor_scalar_mul(hf[:, RO:RO + 1], hf[:, RO:RO + 1], mask_p[:, 0:1])
            nc.vector.tensor_scalar_mul(hf[:, RO + 1:RO + 2], hf[:, RO + 1:RO + 2], mask_n[:, 0:1])
            # depthwise conv along rows: cols 0..511 owned, 512=prev, 513=next.
            # center tap on ACT (scale is per-partition), side taps DVE/GpSimd.
            hfc_t = hfc_pool.tile([128, RO], BF16, name=f"hfc{cc}")
            nc.scalar.activation(hfc_t[:, 0:RO], hf[:, 0:RO], AF.Copy, scale=cw[:, 1:2])
            nc.vector.scalar_tensor_tensor(
                out=hfc_t[:, 1:RO], in0=hf[:, 0:RO - 1], scalar=cw[:, 0:1],
                in1=hfc_t[:, 1:RO], op0=ALU.mult, op1=ALU.add,
            )
            nc.vector.scalar_tensor_tensor(
                out=hfc_t[:, 0:1], in0=hf[:, RO:RO + 1], scalar=cw[:, 0:1],
                in1=hfc_t[:, 0:1], op0=ALU.mult, op1=ALU.add,
            )
            nc.vector.scalar_tensor_tensor(
                out=hfc_t[:, 0:RO - 1], in0=hf[:, 1:RO], scalar=cw[:, 2:3],
                in1=hfc_t[:, 0:RO - 1], op0=ALU.mult, op1=ALU.add,
            )
            nc.vector.scalar_tensor_tensor(
                out=hfc_t[:, RO - 1:RO], in0=hf[:, RO + 1:RO + 2], scalar=cw[:, 2:3],
                in1=hfc_t[:, RO - 1:RO], op0=ALU.mult, op1=ALU.add,
            )
            hfc.append(hfc_t)
    f_stack.close()

    # ---------------- Phase F4: Wdown + residual + output ----------------
    f4_stack = ExitStack()
    wd_pool = f4_stack.enter_context(tc.tile_pool(name="wd", bufs=4))
    out_pool = f4_stack.enter_context(tc.tile_pool(name="outp", bufs=4))
    pp_d = f4_stack.enter_context(tc.tile_pool(name="pp_d", bufs=1, space="PSUM"))
    psd = [[pp_d.tile([128, 512], F32, name=f"psd{i}_{half}") for half in range(2)]
           for i in range(4)]
    for k in range(INNER // 128):
        wd_t = wd_pool.tile([128, D], BF16, tag="wd")
        nc.sync.dma_start(out=wd_t[:], in_=p_wd[k * 128:(k + 1) * 128, :])
        for i in range(4):
            for half in range(2):
                nc.tensor.matmul(
                    psd[i][half][:], hfc[k][:, i * 128:(i + 1) * 128],
                    wd_t[:, half * 512:(half + 1) * 512],
                    start=(k == 0), stop=(k == INNER // 128 - 1),
                )
    for i in range(4):
        o_t = out_pool.tile([128, D], F32, tag="osb")
        for half in range(2):
            s = slice(half * 512, (half + 1) * 512)
            nc.vector.tensor_add(o_t[:, s], psd[i][half][:], qint[i][:, s])
        nc.sync.dma_start(out=p_out[i * 128:(i + 1) * 128, :], in_=o_t[:])
    f4_stack.close()
    f34_stack.close()


def kernel(**inputs) -> np.ndarray:
    Q_in = np.ascontiguousarray(np.asarray(inputs["Q_in"], dtype=np.float32))
    X = np.ascontiguousarray(np.asarray(inputs["X"], dtype=np.float32))
    Wq = np.asarray(inputs["Wq"], dtype=np.float32)
    Wk = np.asarray(inputs["Wk"], dtype=np.float32)
    Wv = np.asarray(inputs["Wv"], dtype=np.float32)
    Wo = np.asarray(inputs["Wo"], dtype=np.float32)
    Wup = np.asarray(inputs["Wup"], dtype=np.float32)
    conv_w = np.asarray(inputs["conv_w"], dtype=np.float32)
    Wdown = np.asarray(inputs["Wdown"], dtype=np.float32)
    g1 = np.asarray(inputs["g1"], dtype=np.float32)
    b1 = np.asarray(inputs["b1"], dtype=np.float32)
    g2 = np.asarray(inputs["g2"], dtype=np.float32)
    b2 = np.asarray(inputs["b2"], dtype=np.float32)
    dt = float(np.asarray(inputs["dt"], dtype=np.float32))

    # softplus(dt) on host; baked into the NEFF as an immediate
    dt_safe = float(np.log1p(np.exp(dt)))

    # fold g2/b2 into Wup (LN2's affine commutes into the up-projection)
    wup_f = g2[:, None] * Wup
    bias_gu = np.ascontiguousarray(b2 @ Wup)
    cw3 = np.ascontiguousarray(conv_w[:, 0, :])

    key = round(dt_safe, 9)
    if key not in _cache:
        _cache[key] = _build(dt_safe)
    nc = _cache[key]

    bf = ml_dtypes.bfloat16
    wq_b = np.ascontiguousarray(Wq.astype(bf))
    wk_b = np.ascontiguousarray(Wk.astype(bf))
    wv_b = np.ascontiguousarray(Wv.astype(bf))
    wo_b = np.ascontiguousarray(Wo.astype(bf))
    wup_b = np.ascontiguousarray(wup_f.astype(bf))
    wd_b = np.ascontiguousarray(Wdown.astype(bf))

    in_maps = []
    for core in range(8):
        b, q = divmod(core, 4)
        qin = np.ascontiguousarray(Q_in[b, q * RO:(q + 1) * RO])
        xb1 = np.ascontiguousarray(X[b, q * RO:(q + 1) * RO] + b1[None, :])
        masks = np.array(
            [1.0 if q > 0 else 0.0, 1.0 if q < NQ - 1 else 0.0], dtype=np.float32
        )
        # maskmat.T @ gathered_boundaries = [prev-halo row; next-halo row]
        mm = np.zeros((2 * NQ, 2), dtype=np.float32)
        if q > 0:
            mm[2 * (q - 1) + 1, 0] = 1.0
        if q < NQ - 1:
            mm[2 * (q + 1), 1] = 1.0
        in_maps.append({
            "q_in": qin, "x_b1": xb1, "wq": wq_b, "wk": wk_b, "wv": wv_b,
            "wo": wo_b, "wup": wup_b, "bias_gu": bias_gu, "wdown": wd_b,
            "cw3": cw3, "g1": np.ascontiguousarray(g1), "masks": masks,
            "maskmat": mm,
        })

    global _last_in_maps
    _last_in_maps = in_maps
    res = run_bass_kernel_spmd(nc, in_maps, core_ids=list(range(8)))

    out = np.empty((B, N, D), dtype=np.float32)
    for core in range(8):
        b, q = divmod(core, 4)
        out[b, q * RO:(q + 1) * RO] = res.results[core]["out"]
    return out
